# revision 1
# baseline (speedup 1.0000x reference)
"""Bass/Tile TRN2 kernel for nn_DiagonalLSTM.

Strategy (see spec sharding_hint): data-parallel over batch. 16 batch
elements across 8 cores -> 2 per core; conv weights replicated.

Per core the computation is a 128-step serial LSTM scan over the skewed
width dimension. The kernel is latency-bound on the loop-carried chain
  matmul -> sigmoid(gates) -> cell update -> tanh(c) -> o*tanh(c) -> matmul
so the design minimizes per-step work on that chain and hides its
latency by running several independent chains (one per batch element,
optionally split again over halves of the row dimension, which only
couples through one ring row) interleaved on the engines:

- All transcendentals are a single Sigmoid table:
    tanh(z) = 2*sigmoid(2z) - 1
  The gate activation uses a per-partition scale vector (1 for o/f/i
  partitions, 2 for the g partitions) so one ACT instruction produces
  sigmoid for three gates and sigmoid(2x) for the candidate gate.
- State is kept in a halved basis to make the *0.5 corrections free:
    CH = c/2,  SH = h/2  (ring storage)
  with W1 pre-scaled by 2 on-device so matmuls consume SH directly,
  and the host multiplies the final output by 2.
- The h-shift (row i sees row i-1 of the previous step) is free: the
  hidden-state ring keeps one zero pad slot in front of each batch
  block, and the "shifted" matmul simply reads the ring at offset -1.
  The same pad trick hands a lower h-half's boundary row to the upper
  half's chain with no extra instructions.
- The input 1x1-conv term is computed per step straight from the
  natural-layout x tile in SBUF using a stride-127 diagonal access
  pattern (engines read strided SBUF at full speed), with a zero-matmul
  covering the not-yet-valid rows i > t.
- Partition-base legality (HW verifier): every 2-input DVE op reads
  both SBUF operands from the same base partition; one realign op per
  step (cgs) plus placing CH at base 32 satisfies it.
"""

import sys

sys.path.insert(0, "/opt/trn_rl_repo")

from contextlib import ExitStack

import numpy as np

import concourse.bass as bass
import concourse.tile as tile
from concourse import bacc, mybir

F32 = mybir.dt.float32
F32R = mybir.dt.float32r
AF = mybir.ActivationFunctionType
ALU = mybir.AluOpType

N_CORES = 8
B = 2  # batch per core
CIN = 32  # input channels
H = 128  # rows (i)
T = 128  # scan steps (skewed column index)
BO = 32  # base_out
G4 = 4 * BO  # gate channels (128)
SLOT = 2 * (H + 1)  # ring slot: [pad, 128 rows] per batch element -> 258
R = 32  # ring depth (slots)
CHUNK = 16  # output DMA chunk, in steps; must divide R and T

H_SPLIT = 2  # chains per batch element (1 or 2): h-halves
POOL_T1 = True  # offload t1 to GPSIMD
POOL_CH = False  # offload ch add to GPSIMD
NBUFS = 2


def _build_module(h_split=None, pool_t1=None, pool_ch=None, nbufs=None, reps=1, t_steps=None, chunk=CHUNK):
    h_split = H_SPLIT if h_split is None else h_split
    pool_t1 = POOL_T1 if pool_t1 is None else pool_t1
    pool_ch = POOL_CH if pool_ch is None else pool_ch
    nbufs = NBUFS if nbufs is None else nbufs
    HC = H // h_split  # rows per chain

    TS = T if t_steps is None else t_steps
    nc = bacc.Bacc(
        "TRN2",
        target_bir_lowering=False,
        debug=False,
        num_devices=N_CORES,
    )

    x_d = nc.dram_tensor("x", [B, CIN, H, T], F32, kind="ExternalInput")
    W2_d = nc.dram_tensor("W2", [G4, CIN], F32, kind="ExternalInput")
    b2_d = nc.dram_tensor("b2", [G4], F32, kind="ExternalInput")
    W1_d = nc.dram_tensor("W1", [G4, BO, 2], F32, kind="ExternalInput")
    b1_d = nc.dram_tensor("b1", [G4], F32, kind="ExternalInput")
    hs_d = nc.dram_tensor("hs", [BO, TS, SLOT], F32, kind="ExternalOutput")

    chains = [(b, h0) for b in range(B) for h0 in range(0, H, HC)]

    with ExitStack() as ctx:
        tc = ctx.enter_context(tile.TileContext(nc))
        const = ctx.enter_context(tc.tile_pool(name="const", bufs=1))
        psum_bufs = 1 if (B * h_split) > 4 else 2
        psum = ctx.enter_context(
            tc.tile_pool(name="psum", bufs=psum_bufs, space="PSUM")
        )
        sig_p = ctx.enter_context(tc.tile_pool(name="sig", bufs=nbufs))
        tmp_p = ctx.enter_context(tc.tile_pool(name="tmp", bufs=nbufs))

        # ---- persistent tiles ----
        xs = const.tile([CIN, B * H * T], F32, tag="xs")  # natural x
        ring = const.tile([BO, R * SLOT], F32, tag="ring")  # SH history
        # CH at partition base 32 so f (= sg[32:64]) pairs legally:
        # the HW verifier demands equal base partitions for two SBUF inputs.
        chbig = {
            c: const.tile([2 * BO, HC], F32, tag=f"ch{i}", name=f"chbig{i}")
            for i, c in enumerate(chains)
        }
        zb = const.tile([2 * BO, 1], F32, tag="zb")  # zero bias (base-32 view)
        w2t = const.tile([CIN, G4], F32, tag="w2t")
    
        w1p = const.tile([BO, G4], F32, tag="w1p")  # 2*W1[:,:,0]^T
        w1c = const.tile([BO, G4], F32, tag="w1c")  # 2*W1[:,:,1]^T
        w1p_raw = const.tile([BO, G4], F32, tag="w1praw")
        w1c_raw = const.tile([BO, G4], F32, tag="w1craw")
        bias = const.tile([G4, 1], F32, tag="bias")
        b2s = const.tile([G4, 1], F32, tag="b2s")
        scale = const.tile([G4, 1], F32, tag="scale")
        zq = const.tile([CIN, HC], F32, tag="zq")  # zero matmul rhs

        # ---- preamble: weights, biases, constants ----
        nc.sync.dma_start(
            out=w2t[:, :], in_=W2_d.ap().rearrange("o c -> c o")
        )
        nc.sync.dma_start(
            out=w1p_raw[:, :], in_=W1_d.ap()[:, :, 0].rearrange("o c -> c o")
        )
        nc.sync.dma_start(
            out=w1c_raw[:, :], in_=W1_d.ap()[:, :, 1].rearrange("o c -> c o")
        )
        nc.sync.dma_start(out=bias[:, :], in_=b1_d.ap()[:, None])
        nc.sync.dma_start(out=b2s[:, :], in_=b2_d.ap()[:, None])
        # x, natural layout: partition=c, free=(b, i, j); one DMA per
        # chain slice so each chain starts as soon as its rows land
        xs4 = xs[:, :].rearrange("c (b i j) -> c b i j", b=B, i=H)
        xd4 = x_d.ap().rearrange("b c i j -> c b i j")
        for b in range(B):
            for h0 in range(0, H, HC):
                nc.sync.dma_start(
                    out=xs4[:, b, h0 : h0 + HC, :],
                    in_=xd4[:, b, h0 : h0 + HC, :],
                )

        nc.vector.memset(zq[:, :], 0.0)
        nc.vector.memset(ring[:, :], 0.0)
        for c in chains:
            nc.vector.memset(chbig[c][:, :], 0.0)
        nc.vector.memset(zb[:, :], 0.0)
        nc.vector.memset(scale[: 3 * BO, :], 1.0)
        nc.vector.memset(scale[3 * BO :, :], 2.0)
        # bias = b1 + b2, then doubled on the g-gate partitions (the ACT
        # computes sigmoid(scale*x + bias), so bias must scale with x).
        nc.vector.tensor_add(bias[:, :], bias[:, :], b2s[:, :])
        nc.vector.tensor_scalar_mul(bias[3 * BO :, :], bias[3 * BO :, :], 2.0)
        # W1 doubled: matmuls read SH = h/2 from the ring.
        nc.vector.tensor_scalar_mul(w1p[:, :], w1p_raw[:, :], 2.0)
        nc.vector.tensor_scalar_mul(w1c[:, :], w1c_raw[:, :], 2.0)

        xv = xs[:, :].rearrange("c (b r) -> c b r", b=B)  # (CIN, B, H*T)
        rv = ring[:, :].rearrange("p (s b q) -> p s b q", s=R, b=B)  # (BO,R,B,H+1)

        # ---- the scan ----
        import contextlib

        rep_ctx = (
            tc.For_i(0, reps, 1) if reps > 1 else contextlib.nullcontext()
        )
        with rep_ctx:
          for t in range(TS):
              sp = (t - 1) % R
              sl = t % R
              # phase-major emission: all chains' matmuls first, then the
              # gate/cell phases, so engine queues interleave chains
              def mm_phase(ci, b, h0):
                  nv = min(t + 1 - h0, HC)
                  g = psum.tile([G4, HC], F32, tag=f"g{ci}", name=f"g{ci}")
                  full = nv == HC
                  has_state = t > 0
                  if not full:
                      nc.tensor.matmul(
                          g[:, :], w2t[:, :], zq[:, :], start=True,
                          stop=not has_state and nv <= 0,
                      )
                  if nv > 0:
                      d0 = t + 127 * h0
                      nc.tensor.matmul(
                          g[:, 0:nv],
                          w2t[:, :],
                          xv[:, b, d0 : d0 + 127 * nv : 127],
                          start=full,
                          stop=not has_state,
                      )
                  if has_state:
                      nc.tensor.matmul(
                          g[:, :],
                          w1p[:, :],
                          rv[:, sp, b, h0 : h0 + HC],
                          start=False,
                          stop=False,
                      )
                      nc.tensor.matmul(
                          g[:, :],
                          w1c[:, :],
                          rv[:, sp, b, h0 + 1 : h0 + 1 + HC],
                          start=False,
                          stop=True,
                      )
                  return g

              def cell_phase(ci, b, h0, g):
                  sg = sig_p.tile([G4, HC], F32, tag=f"sg{ci}", name=f"sg{ci}")
                  nc.scalar.activation(
                      sg[:, :], g[:, :], AF.Sigmoid, bias=bias[:, :],
                      scale=scale[:, :],
                  )
                  cgb = tmp_p.tile([3 * BO, HC], F32, tag=f"cgb{ci}",
                                   name=f"cgb{ci}")
                  cgs = cgb[2 * BO : 3 * BO, :]
                  u = tmp_p.tile([BO, HC], F32, tag=f"u{ci}", name=f"u{ci}")
                  t1 = tmp_p.tile([BO, HC], F32, tag=f"t1{ci}", name=f"t1{ci}")
                  ch = chbig[(b, h0)][BO : 2 * BO, :]
                  nc.vector.tensor_scalar_sub(cgs, sg[3 * BO : 4 * BO, :], 0.5)
                  nc.vector.tensor_tensor(
                      u[:, :], cgs, sg[2 * BO : 3 * BO, :], ALU.mult
                  )
                  eng_t1 = nc.gpsimd if pool_t1 else nc.vector
                  eng_t1.tensor_tensor(t1[:, :], sg[BO : 2 * BO, :], ch, ALU.mult)
                  eng_ch = nc.gpsimd if pool_ch else nc.vector
                  eng_ch.tensor_tensor(ch, t1[:, :], u[:, :], ALU.add)
                  tch = tmp_p.tile([BO, HC], F32, tag=f"tch{ci}", name=f"tch{ci}")
                  nc.scalar.activation(
                      tch[:, :], ch, AF.Sigmoid, bias=zb[BO : 2 * BO, :],
                      scale=4.0,
                  )
                  nc.vector.scalar_tensor_tensor(
                      rv[:, sl, b, h0 + 1 : h0 + 1 + HC],
                      tch[:, :],
                      0.5,
                      sg[0:BO, :],
                      ALU.subtract,
                      ALU.mult,
                  )

              gs = [mm_phase(ci, b, h0) for ci, (b, h0) in enumerate(chains)]
              for ci, (b, h0) in enumerate(chains):
                  cell_phase(ci, b, h0, gs[ci])

              # stream finished slots out
              if t % chunk == chunk - 1:
                  c0 = t - chunk + 1
                  s0 = c0 % R
                  nc.sync.dma_start(
                      out=hs_d.ap()[:, c0 : t + 1, :],
                      in_=ring[:, s0 * SLOT : (s0 + chunk) * SLOT],
                  )

    nc.compile()
    return nc


_NC_CACHE = {}


def _get_module(**kw):
    key = tuple(sorted(kw.items()))
    if key not in _NC_CACHE:
        _NC_CACHE[key] = _build_module(**kw)
    return _NC_CACHE[key]


def kernel(x, W2, b2, W1, b1):
    from concourse.bass_utils import run_bass_kernel_spmd

    nc = _get_module()
    x = np.ascontiguousarray(x, dtype=np.float32)
    in_maps = [
        {
            "x": x[2 * k : 2 * k + 2],
            "W2": np.asarray(W2, np.float32),
            "b2": np.asarray(b2, np.float32),
            "W1": np.asarray(W1, np.float32),
            "b1": np.asarray(b1, np.float32),
        }
        for k in range(N_CORES)
    ]
    res = run_bass_kernel_spmd(nc, in_maps, list(range(N_CORES)))
    out = np.empty((N_CORES * B, BO, H, T), np.float32)
    for k in range(N_CORES):
        hs = res.results[k]["hs"]  # (BO, T, SLOT)
        v = hs.reshape(BO, T, B, H + 1)[:, :, :, 1:]  # (o, t, b, i)
        out[B * k : B * k + B] = 2.0 * v.transpose(2, 0, 3, 1)
    return out

